# revision 15
# baseline (speedup 1.0000x reference)
"""
Trainium2 Bass kernel for DynamicGraphAttention
(softmax(Hn Wq^T (Hn Wk^T)^T / sqrt(D) + eta*logit(clip(A)) masked)).

Shapes (hardcoded):
  Hn     [16, 2048, 256] f32
  A_stat [2048, 2048]    f32
  M_mask [2048, 2048]    int32
  Wq, Wk [256, 256]      f32
  out    [16, 2048, 2048] f32

Sharding across 8 NeuronCores: 2 batch-groups x 4 seq(query)-groups.
Core c handles batches [bg*8:(bg+1)*8] (bg = c // 4) and query rows
[qg*512:(qg+1)*512] (qg = c % 4). A_stat/M_mask are row-sharded by the
query group; Hn is replicated within a batch group (the key side needs
all nodes).

Device algorithm (per core):
  G   = (Wq^T Wk) / sqrt(D)                      [256,256]    (TensorE)
  B   = logit(clip(A)) + mask  (split bf16 hi/lo)             (DVE+ACT)
  VT  = G^T HnT[:, qwin]   per batch             [256,512]    (TensorE)
  S   = VT.T @ HnT (+ B via identity matmuls)    PSUM         (TensorE)
  P   = exp(S), rowsum via accum_out             (ScalarE)
  out = P * (1/rowsum)                           (DVE), DMA out
"""

import math

import numpy as np

import concourse.bass as bass
import concourse.bacc as bacc
import concourse.tile as tile
from concourse import mybir
from concourse import bass_utils

F32 = mybir.dt.float32
F32R = mybir.dt.float32r
BF16 = mybir.dt.bfloat16
FP16 = mybir.dt.float16

B_FULL = 16
N = 2048
D = 256
NBG = 2   # batch groups
NQG = 4   # seq (query-row) groups
NB = B_FULL // NBG        # batches per core = 8
NQ = N // NQG             # query rows per core = 512
NQT = NQ // 128           # q tiles per core-batch = 4
EPS = 1e-3
TINY = 1e-30
SCALE = 1.0 / math.sqrt(float(D))  # 1/16

_CACHE = {}


def _build():
    nc = bacc.Bacc("TRN2", debug=False, enable_asserts=False)

    hnt_d = nc.dram_tensor("hnt", [NB, D, N], FP16, kind="ExternalInput").ap()
    hqt_d = nc.dram_tensor("hqt", [NB, D, NQ], FP16, kind="ExternalInput").ap()
    a_d = nc.dram_tensor("a", [NQ, N], F32, kind="ExternalInput").ap()
    m_d = nc.dram_tensor("m", [NQ, N], BF16, kind="ExternalInput").ap()
    wq_d = nc.dram_tensor("wq", [D, D], F32, kind="ExternalInput").ap()
    wk_d = nc.dram_tensor("wk", [D, D], F32, kind="ExternalInput").ap()
    idb_d = nc.dram_tensor("idb", [128, 128], FP16, kind="ExternalInput").ap()
    o_d = nc.dram_tensor("o", [NB, NQ, N], F32, kind="ExternalOutput").ap()

    with tile.TileContext(nc) as tc:
        with (
            tc.tile_pool(name="consts", bufs=1) as consts,
            tc.tile_pool(name="prep", bufs=2) as prep,
            tc.tile_pool(name="bpool", bufs=1) as bpool,
            tc.tile_pool(name="hntp", bufs=16) as hntp,
            tc.tile_pool(name="hqtp", bufs=6) as hqtp,
            tc.tile_pool(name="vtp", bufs=16) as vtp,
            tc.tile_pool(name="pp", bufs=4) as pp,
            tc.tile_pool(name="rsp", bufs=8) as rsp,
            tc.tile_pool(name="ps_s", bufs=2, space="PSUM") as ps_s,
        ):
            # ---- constants ----
            idb = consts.tile([128, 128], FP16, tag="idb")
            nc.sync.dma_start(out=idb, in_=idb_d)
            wq_sb = consts.tile([128, 2, D], F32, tag="wq")
            nc.sync.dma_start(out=wq_sb, in_=wq_d.rearrange("(c p) d -> p c d", p=128))
            wk_sb = consts.tile([128, 2, D], F32, tag="wk")
            nc.sync.dma_start(out=wk_sb, in_=wk_d.rearrange("(c p) d -> p c d", p=128))
            tinyc = consts.tile([128, 1], F32, tag="tiny")
            nc.vector.memset(tinyc, float(TINY))

            # ---- G = (Wq^T Wk) * SCALE : [256, 256] as 2 tiles [128(i), 256(j)] ----
            g = []
            for i in range(2):
                gp = ps_s.tile([128, N], F32, tag="s", name=f"gp{i}")
                for e in range(2):
                    nc.tensor.matmul(
                        gp[:, :D],
                        lhsT=wq_sb[:, e, i * 128:(i + 1) * 128],
                        rhs=wk_sb[:, e, :],
                        start=(e == 0),
                        stop=(e == 1),
                    )
                g_i = consts.tile([128, D], FP16, tag=f"g{i}", name=f"g{i}")
                nc.scalar.mul(out=g_i, in_=gp[:, :D], mul=SCALE)
                g.append(g_i)

            # ---- B table prep: 4 tiles of [128, N] (bf16 hi+lo), built in halves --
            bhi = []
            blo = []
            for t in range(NQT):
                bhi_t = bpool.tile([128, N], FP16, tag=f"bhi{t}", name=f"bhi{t}")
                blo_t = bpool.tile([128, N], FP16, tag=f"blo{t}", name=f"blo{t}")
                bhi.append(bhi_t)
                blo.append(blo_t)
                for h in range(2):
                    sl = slice(h * 1024, (h + 1) * 1024)
                    a_t = prep.tile([128, 1024], F32, tag="a", name=f"a{t}{h}")
                    nc.sync.dma_start(out=a_t, in_=a_d[t * 128:(t + 1) * 128, sl])
                    m_t = prep.tile([128, 1024], BF16, tag="m", name=f"m{t}{h}")
                    nc.sync.dma_start(out=m_t, in_=m_d[t * 128:(t + 1) * 128, sl])
                    # clip to [EPS, 1-EPS] (in place)
                    nc.vector.tensor_scalar(
                        out=a_t, in0=a_t, scalar1=float(EPS),
                        scalar2=float(1.0 - EPS),
                        op0=mybir.AluOpType.max, op1=mybir.AluOpType.min,
                    )
                    # apply mask multiplicatively: masked entries -> 0
                    nc.vector.tensor_mul(a_t, a_t, m_t)
                    # la = ln(a + TINY); l1a = ln(1 - a)
                    la = prep.tile([128, 1024], F32, tag="la", name=f"la{t}{h}")
                    nc.scalar.activation(
                        out=la, in_=a_t, func=mybir.ActivationFunctionType.Ln,
                        bias=tinyc, scale=1.0,
                    )
                    l1a = prep.tile([128, 1024], F32, tag="l1a", name=f"l1a{t}{h}")
                    nc.scalar.activation(
                        out=l1a, in_=a_t, func=mybir.ActivationFunctionType.Ln,
                        bias=1.0, scale=-1.0,
                    )
                    # B = la - l1a  (f32, into la); split into bf16 hi + lo
                    nc.vector.tensor_sub(la, la, l1a)
                    nc.vector.tensor_copy(out=bhi_t[:, sl], in_=la)
                    nc.vector.tensor_sub(blo_t[:, sl], la, bhi_t[:, sl])

            # ---- VT for all batches up front (fills PE while B-prep runs) --
            vts = []
            for b in range(NB):
                hqt = []
                for i in range(2):
                    hq_i = hqtp.tile([128, NQ], FP16, tag="hqt", name=f"hqt{b}_{i}")
                    nc.sync.dma_start(
                        out=hq_i, in_=hqt_d[b, i * 128:(i + 1) * 128, :]
                    )
                    hqt.append(hq_i)
                vt = []
                for j in range(2):
                    vt_j = vtp.tile([128, NQ], FP16, tag="vt", name=f"vt{b}_{j}")
                    for c in range(NQ // 512):
                        csl = slice(c * 512, (c + 1) * 512)
                        vp = ps_s.tile(
                            [128, N], F32, tag="s", name=f"vp{b}{j}{c}"
                        )[:, :512]
                        for i in range(2):
                            nc.tensor.matmul(
                                vp,
                                lhsT=g[i][:, j * 128:(j + 1) * 128],
                                rhs=hqt[i][:, csl],
                                start=(i == 0),
                                stop=(i == 1),
                            )
                        nc.scalar.copy(out=vt_j[:, csl], in_=vp)
                    vt.append(vt_j)
                vts.append(vt)

            # ---- per batch ----
            for b in range(NB):
                vt = vts[b]
                hnt = []
                for i in range(2):
                    h_i = hntp.tile([128, N], FP16, tag="hnt", name=f"hnt{b}_{i}")
                    nc.sync.dma_start(
                        out=h_i, in_=hnt_d[b, i * 128:(i + 1) * 128, :]
                    )
                    hnt.append(h_i)

                # S-loop over q tiles: one full-width PSUM tile per q tile
                for qt in range(NQT):
                    qsl = slice(qt * 128, (qt + 1) * 128)
                    s_ps = ps_s.tile([128, N], F32, tag="s", name=f"s{b}{qt}")
                    for j in range(2):
                        for c in range(4):
                            csl = slice(c * 512, (c + 1) * 512)
                            nc.tensor.matmul(
                                s_ps[:, csl],
                                lhsT=vt[j][:, qsl],
                                rhs=hnt[j][:, csl],
                                start=(j == 0),
                                stop=False,
                            )
                    for c in range(4):
                        csl = slice(c * 512, (c + 1) * 512)
                        nc.tensor.matmul(
                            s_ps[:, csl], lhsT=idb, rhs=bhi[qt][:, csl],
                            start=False, stop=False,
                        )
                    for c in range(4):
                        csl = slice(c * 512, (c + 1) * 512)
                        nc.tensor.matmul(
                            s_ps[:, csl], lhsT=idb, rhs=blo[qt][:, csl],
                            start=False, stop=True,
                        )
                    p = pp.tile([128, N], F32, tag="p", name=f"p{b}{qt}")
                    rs = rsp.tile([128, 1], F32, tag="rs", name=f"rs{b}{qt}")
                    nc.scalar.activation(
                        out=p, in_=s_ps,
                        func=mybir.ActivationFunctionType.Exp,
                        accum_out=rs,
                    )
                    rinv = rsp.tile([128, 1], F32, tag="rinv", name=f"ri{b}{qt}")
                    nc.vector.reciprocal(out=rinv, in_=rs)
                    nc.vector.tensor_scalar(
                        out=p, in0=p, scalar1=rinv, scalar2=None,
                        op0=mybir.AluOpType.mult,
                    )
                    nc.gpsimd.dma_start(out=o_d[b, qsl, :], in_=p)
    nc.compile()
    return nc


def _get_nc():
    if "nc" not in _CACHE:
        _CACHE["nc"] = _build()
    return _CACHE["nc"]


def make_in_maps(Hn, A_stat, M_mask, Wq, Wk):
    import ml_dtypes

    Hn = np.ascontiguousarray(np.asarray(Hn, dtype=np.float32))
    A_stat = np.ascontiguousarray(np.asarray(A_stat, dtype=np.float32))
    M_mask = np.asarray(M_mask)
    Wq = np.ascontiguousarray(np.asarray(Wq, dtype=np.float32))
    Wk = np.ascontiguousarray(np.asarray(Wk, dtype=np.float32))
    assert Hn.shape == (B_FULL, N, D)

    m_bf16 = M_mask.astype(np.float32).astype(ml_dtypes.bfloat16)
    idb = np.eye(128, dtype=np.float16)

    # [16, 256, 2048] transposed-node layout, fp16 (the PE's reduced
    # precision matmul formats carry ~10 mantissa bits anyway)
    hnt_full = np.ascontiguousarray(Hn.astype(np.float16).transpose(0, 2, 1))

    in_maps = []
    for c in range(8):
        bg, qg = c // NQG, c % NQG
        bsl = slice(bg * NB, (bg + 1) * NB)
        qsl = slice(qg * NQ, (qg + 1) * NQ)
        in_maps.append({
            "hnt": hnt_full[bsl],
            "hqt": np.ascontiguousarray(hnt_full[bsl][:, :, qsl]),
            "a": A_stat[qsl],
            "m": np.ascontiguousarray(m_bf16[qsl]),
            "wq": Wq,
            "wk": Wk,
            "idb": idb,
        })
    return in_maps


def assemble(results):
    out = np.empty((B_FULL, N, N), dtype=np.float32)
    for c in range(8):
        bg, qg = c // NQG, c % NQG
        o = results[c]["o"]
        out[bg * NB:(bg + 1) * NB, qg * NQ:(qg + 1) * NQ, :] = o
    return out


def kernel(Hn, A_stat, M_mask, Wq, Wk):
    in_maps = make_in_maps(Hn, A_stat, M_mask, Wq, Wk)
    nc = _get_nc()
    res = bass_utils.run_bass_kernel_spmd(nc, in_maps, core_ids=list(range(8)))
    return assemble(res.results)


if __name__ == "__main__":
    rng = np.random.default_rng(0)
    inputs = {
        "Hn": rng.standard_normal((B_FULL, N, D), dtype=np.float32),
        "A_stat": rng.random((N, N), dtype=np.float32),
        "M_mask": rng.integers(0, 2, size=(N, N), dtype=np.int32),
        "Wq": rng.standard_normal((D, D), dtype=np.float32) / 16,
        "Wk": rng.standard_normal((D, D), dtype=np.float32) / 16,
    }
    out = kernel(**inputs)
    print(out.shape, out.dtype, out.sum())
